# revision 2
# baseline (speedup 1.0000x reference)
"""Trainium2 Bass kernel for nn_AugmentableSVSAlgorithm (scatter_memory) — v2.

Same math/layout as the v1 tile kernel (see kernel.py docstring): 8-core
H-split SPMD, 126 partitions = (h:18, wb:7), pages of 25 cols, x = +1e6
padding saturates the sigmoids so hot == 1 in halos.

v2 is raw bass (no TileContext) with hand-placed semaphores, built around a
FUSED chain+eps custom DVE op:

  * One DVE op per time step processes [P, 2, 50]: page 0 advances the
    recurrence (select(u>0, S+d_open, S-d_close), u = x2-S, bit-exact vs
    the reference since sign(fl(x-S)) == sign(x-S) and S+(-b) == S-b),
    page 1 emits u itself — the sigmoid argument — into the same ring
    slot.  The eps half also spaces consecutive ops' SBUF writes/reads, so
    NO per-step semaphore is needed (back-to-back ops with <~60 elems race
    on write commit; 100 elems is safely past the cliff measured at 10).
  * traj is a 200-slot ring of [S' | eps] pairs; no carry copies.
  * ScalarE: th = sigmoid(hc*eps - hc*d_hot) from the eps pages, fp16.
  * GpSimd pre-sums hot = th + tl (fp16, 25 elems/step - light traffic on
    the DVE-shared SBUF ports).
  * TensorE: 3 banded matmuls per 20-step chunk (dy via stationary band,
    dx via shifted moving windows over hot's 25-col pages).
  * ScalarE relu(ps + (1-ksum)) -> outw; per-window DMA out.
"""

import numpy as np

T, H, W = 2000, 128, 160
NCORES = 8
ROWS = 18
NB = 7
PW = 25
STRIDE = 23
P = ROWS * NB        # 126
FD = 2 * PW          # 50 state elems per step
SLOT = 2 * FD        # ring slot: [S' (50) | eps (50)]
RS = 200             # traj ring slots
WIN = 100
CE = 10              # sigmoid/hot chunk (steps)
CH = 20              # conv/relu chunk (steps)
PAD = np.float32(1.0e6)

WINS = [20, 40, 40] + [100] * 18 + [40, 40, 20]
assert sum(WINS) == T and all(w % CH == 0 for w in WINS)

PS_RING = 3          # conv PSUM banks
TH_RING = 16         # th SBUF chunk slots

_OP = None


def _register_op():
    """Fused step: page 0 -> state update, page 1 -> eps = x2 - S."""
    global _OP
    if _OP is not None:
        return _OP
    from concourse import dve_ops
    from concourse.dve_spec import (
        Spec, Src0, Src1, C0, C1, SubIdx, Zero, select, lower,
    )
    from concourse.dve_uop import DveOpSpec

    name = "SVS_STEP_EPS_ANT"
    for o in dve_ops.OPS:
        if o.name == name:
            _OP = o
            return o

    def _ref(in0, in1, c0, c1, c2):
        u = (in0 - in1).astype(np.float32)
        chain = np.where(u > 0, (in1 + np.float32(c0)).astype(np.float32),
                         (in1 - np.float32(c1)).astype(np.float32))
        out = chain.astype(np.float32)
        out[:, 1, :] = u[:, 1, :]
        return out

    u = Src0 - Src1
    spec = Spec(
        body=select(SubIdx, u, Src1 + select(Zero < u, C0, Zero - C1)),
        reference=_ref,
    )
    opcode = dve_ops._CUSTOM_DVE_ROW_BASE + len(dve_ops.OPS)
    shas = {}
    for ver in ("v3", "v4"):
        uops = lower(spec, ver=ver)
        shas[ver] = DveOpSpec(name=name, opcode=opcode, uops=uops, rd1_en=True).sha(ver)
    op = dve_ops.DveOp(name, spec, subdim=True, uops_sha=shas)
    dve_ops.OPS.append(op)
    dve_ops._SUB_OPCODE_FOR_NAME[name] = opcode
    dve_ops.CUSTOM_DVE_SPECS[name] = spec
    _OP = op
    return op


def _build_program(d_open, d_close, hc, sig_bias, relu_bias):
    from concourse import mybir, bacc

    op = _register_op()
    nc = bacc.Bacc("TRN2", target_bir_lowering=False, debug=False,
                   num_devices=NCORES)
    f32 = mybir.dt.float32
    f16 = mybir.dt.float16
    Sig = mybir.ActivationFunctionType.Sigmoid
    Relu = mybir.ActivationFunctionType.Relu

    xp_d = nc.dram_tensor("xp", [P, T, FD], f32, kind="ExternalInput").ap()
    s0_d = nc.dram_tensor("s0", [P, FD], f32, kind="ExternalInput").ap()
    band_d = nc.dram_tensor("band", [3, P, P], f16, kind="ExternalInput").ap()
    out_d = nc.dram_tensor("out", [P, T, STRIDE], f32, kind="ExternalOutput").ap()

    # SBUF
    x2b = nc.alloc_sbuf_tensor("x2b", [P, 3 * WIN * FD], f32).ap()
    trajr = nc.alloc_sbuf_tensor("trajr", [P, RS * SLOT], f32).ap()
    bands = nc.alloc_sbuf_tensor("bands", [P, 3 * P], f16).ap()
    thr = nc.alloc_sbuf_tensor("thr", [P, TH_RING * CE * FD], f16).ap()
    outw = nc.alloc_sbuf_tensor("outw", [P, 2 * STRIDE * WIN], f32).ap()
    # PSUM
    psb = [nc.alloc_psum_tensor(f"psb{i}", [P, CH * STRIDE], f32).ap()
           for i in range(PS_RING)]

    # semaphores
    sem_const = nc.alloc_semaphore("sem_const")  # bands+s0 DMAs (2 x 16)
    sem_x2 = [nc.alloc_semaphore("sem_x2a"),
              nc.alloc_semaphore("sem_x2b"),
              nc.alloc_semaphore("sem_x2c")]     # x2 buffer mod-3
    sem_chain = nc.alloc_semaphore("sem_chain")  # +1 per 10-step chain chunk
    sem_sig = nc.alloc_semaphore("sem_sig")      # +1 per sigmoid chunk
    sem_ps = nc.alloc_semaphore("sem_ps")        # +1 per conv chunk (TensorE)
    sem_relu = nc.alloc_semaphore("sem_relu")    # +1 per relu chunk (ScalarE)
    sem_out = [nc.alloc_semaphore("sem_outa"),
               nc.alloc_semaphore("sem_outb")]   # out DMA parity
    sem_cb = nc.alloc_semaphore("sem_cb")        # bias memsets done

    sig_bias_t = nc.alloc_sbuf_tensor("sig_bias_t", [P, 1], f32).ap()
    relu_bias_t = nc.alloc_sbuf_tensor("relu_bias_t", [P, 1], f32).ap()
    nc.gpsimd.memset(sig_bias_t[:], sig_bias)
    nc.gpsimd.memset(relu_bias_t[:], relu_bias).then_inc(sem_cb, 16)
    nc.scalar.wait_ge(sem_cb, 16)

    # window bookkeeping
    t0s = []
    t0 = 0
    for wlen in WINS:
        t0s.append(t0)
        t0 += wlen
    cum10 = np.cumsum([w // CE for w in WINS]).tolist()
    cum20 = np.cumsum([w // CH for w in WINS]).tolist()
    NW = len(WINS)
    NC20 = cum20[-1]

    x2v = x2b.rearrange("p (b t f) -> p b t f", b=3, f=FD)       # [P,3,WIN,FD]
    trajv = trajr.rearrange("p (s h f) -> p s h f", s=RS, f=FD)  # [P,RS,2,FD]
    thv = thr.rearrange("p (r t f) -> p r t f", r=TH_RING, f=FD)
    outwv = outw.rearrange("p (b j) -> p b j", b=2)

    # ---- const / initial DMAs (sync queue) ----
    nc.sync.dma_start(bands[:].rearrange("p (d q) -> p d q", d=3),
                      band_d.rearrange("d p q -> p d q")).then_inc(sem_const, 16)
    # s0 lives in the state half of ring slot RS-1 (step 0 reads it there)
    nc.sync.dma_start(trajv[:, RS - 1, 0, :], s0_d[:]).then_inc(sem_const, 16)

    def x2_dma(w):
        wlen = WINS[w]
        if w >= 3:
            # x2 buffer WAR: the chain (its only reader) is done with window
            # w-3 once sigmoid is, and gating on sigmoid also bounds how far
            # the chain can run ahead of the eps consumers (ring overrun)
            nc.sync.wait_ge(sem_sig, cum10[w - 3])
        nc.sync.dma_start(
            x2v[:, w % 3, 0:wlen, :],
            xp_d[:, t0s[w]:t0s[w] + wlen, :],
        ).then_inc(sem_x2[w % 3], 16)

    x2_dma(0)
    x2_dma(1)
    x2_dma(2)

    # ---- engine emission helpers ----
    def chain_window(w):
        wlen = WINS[w]
        t0w = t0s[w]
        if w == 0:
            nc.vector.wait_ge(sem_const, 32)
        nc.vector.wait_ge(sem_x2[w % 3], 16 * (w // 3 + 1))
        for i in range(wlen):
            t = t0w + i
            sprev = (t - 1) % RS
            inst = nc.vector._custom_dve(
                op,
                out=trajv[:, t % RS, :, :],
                in0=x2v[:, w % 3, i, :].unsqueeze(1).broadcast_to((P, 2, FD)),
                in1=trajv[:, sprev, 0, :].unsqueeze(1).broadcast_to((P, 2, FD)),
                s0=d_open,
                s1=d_close,
            )
            if (t + 1) % CE == 0:
                inst.then_inc(sem_chain, 1)

    def sig_chunk(k):
        # eps pages of steps [10k, 10k+10) live at ring slots 10k..10k+9
        s1 = (CE * k) % RS
        nc.scalar.wait_ge(sem_chain, k + 1)
        if k >= TH_RING:
            nc.scalar.wait_ge(sem_ps, (k - TH_RING) // 2 + 1)
        nc.scalar.activation(
            thv[:, k % TH_RING, :, :], trajv[:, s1:s1 + CE, 1, :],
            Sig, bias=sig_bias_t[:], scale=hc,
        ).then_inc(sem_sig, 1)

    def conv_chunk(j):
        # steps [CH*j, CH*j+CH) = th chunks 2j, 2j+1 (contiguous ring slots)
        if j == 0:
            nc.tensor.wait_ge(sem_const, 32)
        nc.tensor.wait_ge(sem_sig, 2 * j + 2)
        if j >= PS_RING:
            nc.tensor.wait_ge(sem_relu, j - PS_RING + 1)
        ps = psb[j % PS_RING][:].rearrange("p (t f) -> p t f", t=CH)
        h0 = (2 * j) % TH_RING
        th2 = thr[:, h0 * CE * FD:(h0 + 2) * CE * FD].rearrange(
            "p (t f) -> p t f", f=FD)
        m = 0
        for dx in (-1, 0, 1):
            for half in (0, PW):
                o = half + 1 + dx
                inst = nc.tensor.matmul(
                    ps, bands[:, (dx + 1) * P:(dx + 2) * P],
                    th2[:, :, o:o + STRIDE],
                    start=(m == 0), stop=(m == 5),
                )
                m += 1
        inst.then_inc(sem_ps, 1)

    def relu_chunk(j):
        w = next(i for i, c in enumerate(cum20) if j < c)
        first_of_w = (j == 0) or (j == cum20[w - 1])
        if first_of_w and w >= 2:
            nc.scalar.wait_ge(sem_out[w % 2], 16 * (w // 2))
        nc.scalar.wait_ge(sem_ps, j + 1)
        off = (CH * j - t0s[w]) * STRIDE
        nc.scalar.activation(
            outwv[:, w % 2, off:off + CH * STRIDE], psb[j % PS_RING][:],
            Relu, bias=relu_bias_t[:], scale=1.0,
        ).then_inc(sem_relu, 1)

    def out_dma(w):
        wlen = WINS[w]
        nc.sync.wait_ge(sem_relu, cum20[w])
        nc.sync.dma_start(
            out_d[:, t0s[w]:t0s[w] + wlen, :],
            outwv[:, w % 2, 0:STRIDE * wlen].rearrange(
                "p (t f) -> p t f", f=STRIDE),
        ).then_inc(sem_out[w % 2], 16)

    # ---- emit (order matters only within each engine's own queue) ----
    for w in range(NW):
        chain_window(w)
        k_lo = 0 if w == 0 else cum10[w - 1]
        for k in range(k_lo, cum10[w]):
            sig_chunk(k)
            if k % 2 == 1:
                conv_chunk((k - 1) // 2)
                # relu j lands two sigmoid chunks after conv j so ScalarE
                # doesn't stall on the PE
                if k >= 3:
                    relu_chunk((k - 3) // 2)
        if w + 3 < NW:
            x2_dma(w + 3)
        out_dma(w)
    relu_chunk(NC20 - 1)

    nc.sync.wait_ge(sem_out[0], 16 * ((NW + 1) // 2))
    nc.sync.wait_ge(sem_out[1], 16 * (NW // 2))
    nc.all_engine_barrier()
    nc.compile()
    return nc


_PROG_CACHE = {}


def _get_program(key, d_open, d_close, hc, sig_bias, relu_bias):
    if key not in _PROG_CACHE:
        _PROG_CACHE[key] = _build_program(d_open, d_close, hc, sig_bias,
                                          relu_bias)
    return _PROG_CACHE[key]


def _prep_inputs(x, ht0, lt0, kern, hc):
    x = np.ascontiguousarray(x.reshape(T, H, W).astype(np.float32))
    ht0 = ht0.astype(np.float32)
    lt0 = lt0.astype(np.float32)
    kern = kern.astype(np.float32)

    xp = np.full((T, H + 2, W + 3), PAD, np.float32)
    xp[:, 1:H + 1, 1:W + 1] = x
    hp = np.zeros((H + 2, W + 3), np.float32)
    hp[1:H + 1, 1:W + 1] = ht0
    lp = np.zeros((H + 2, W + 3), np.float32)
    lp[1:H + 1, 1:W + 1] = -lt0

    band = np.zeros((3, P, P), np.float16)
    for dxi in range(3):
        for h_out in range(ROWS):
            for dy in (-1, 0, 1):
                h_in = h_out + dy
                if 0 <= h_in < ROWS:
                    for wb in range(NB):
                        band[dxi, h_in * NB + wb, h_out * NB + wb] = \
                            kern[dy + 1, dxi]

    in_maps = []
    for c in range(NCORES):
        r0 = 16 * c
        xc = np.empty((ROWS, NB, T, FD), np.float32)
        sc = np.empty((ROWS, NB, FD), np.float32)
        for wb in range(NB):
            c0 = STRIDE * wb
            blk = xp[:, r0:r0 + ROWS, c0:c0 + PW].transpose(1, 0, 2)
            xc[:, wb, :, 0:PW] = blk
            xc[:, wb, :, PW:FD] = -blk
            sc[:, wb, 0:PW] = hp[r0:r0 + ROWS, c0:c0 + PW]
            sc[:, wb, PW:FD] = lp[r0:r0 + ROWS, c0:c0 + PW]
        in_maps.append({
            "xp": np.ascontiguousarray(xc.reshape(P, T, FD)),
            "s0": np.ascontiguousarray(sc.reshape(P, FD)),
            "band": band,
        })
    return in_maps


TRACE = False
LAST_RESULT = None


def kernel(x, params, ht0, lt0, kernel):
    global LAST_RESULT
    from concourse.bass_utils import run_bass_kernel_spmd

    p = np.asarray(params, np.float32)
    d_close, d_open, d_hot, hc = (float(p[0]), float(p[1]), float(p[2]),
                                  float(p[3]))
    kern = np.asarray(kernel, np.float32)
    sig_bias = float(np.float32(-np.float32(d_hot) * np.float32(hc)))
    relu_bias = float(np.float32(1.0) - np.float32(kern.sum()))

    key = (d_close, d_open, d_hot, hc, kern.tobytes())
    nc = _get_program(key, d_open, d_close, hc, sig_bias, relu_bias)
    in_maps = _prep_inputs(np.asarray(x), np.asarray(ht0), np.asarray(lt0),
                           kern, hc)
    r = run_bass_kernel_spmd(nc, in_maps, list(range(NCORES)), trace=TRACE)
    LAST_RESULT = r
    res = r.results
    out = np.empty((T, H, W), np.float32)
    for c in range(NCORES):
        out[:, 16 * c:16 * (c + 1), :] = _assemble(res[c]["out"])
    return out.reshape(T, 1, H, W).astype(np.float32)


def _assemble(raw):
    v = raw.reshape(ROWS, NB, T, STRIDE)[1:17]
    full = v.transpose(2, 0, 1, 3).reshape(T, 16, NB * STRIDE)
    return full[:, :, :W]


# revision 3
# speedup vs baseline: 1.0000x; 1.0000x over previous
"""Trainium2 Bass kernel for nn_AugmentableSVSAlgorithm (scatter_memory) — v2.

Reference semantics: per-frame recurrence over T=2000 frames with carry
(ht, lt) [128,160]; th/tl sigmoids against the PRE-update thresholds; then
out = relu(1 - conv3x3(1 - (th+tl), pad=1)).

Layout: 8-core H-split SPMD (core c owns rows [16c,16c+16), processes
[16c-1,16c+17) with halos).  126 partitions = (h:18, wb:7), pages of 25
cols (23-col stride + 2 halo cols).  Out-of-range halo rows/cols carry
x = +1e6, which saturates both sigmoids so hot == 1 exactly, reproducing
the reference conv's zero-padding of (1-hot) with no edge cases.  State
pairs S = [h | L], L = -lt, share one update with x2 = [x | -x].

Raw bass (no TileContext) with hand-placed semaphores, built around a
FUSED chain+eps custom DVE op:

  * One DVE op per time step processes [P, 2, 50]: page 0 advances the
    recurrence (select(u>0, S+d_open, S-d_close), u = x2-S, bit-exact vs
    the reference since sign(fl(x-S)) == sign(x-S) and S+(-b) == S-b),
    page 1 emits u itself — the sigmoid argument — into the same ring
    slot.  The eps half also spaces consecutive ops' SBUF writes/reads, so
    NO per-step semaphore is needed (back-to-back ops with <~60 elems race
    on write commit; 100 elems is safely past the cliff measured at 10).
  * traj is a 200-slot ring of [S' | eps] pairs; no carry copies.
  * ScalarE: th = sigmoid(hc*eps - hc*d_hot) from the eps pages, fp16.
  * TensorE: 6 banded fp16 matmuls per 20-step chunk (dy via stationary
    band, dx via shifted moving windows, th/tl halves summed by PSUM
    accumulation).  GpSimd is unused (~2.2us fixed cost per op makes it
    useless for small chunks, and it contends with the DVE's SBUF ports).
  * ScalarE relu(ps + (1-ksum)) -> outw; per-window DMA out.
"""

import numpy as np

T, H, W = 2000, 128, 160
NCORES = 8
ROWS = 18
NB = 7
PW = 25
STRIDE = 23
P = ROWS * NB        # 126
FD = 2 * PW          # 50 state elems per step
SLOT = 2 * FD        # ring slot: [S' (50) | eps (50)]
RS = 200             # traj ring slots
WIN = 100
CE = 10              # sigmoid/hot chunk (steps)
CH = 20              # conv/relu chunk (steps)
PAD = np.float32(1.0e6)

WINS = [20, 40, 40] + [100] * 18 + [40, 40, 20]
assert sum(WINS) == T and all(w % CH == 0 for w in WINS)

PS_RING = 3          # conv PSUM banks
TH_RING = 16         # th SBUF chunk slots

_OP = None


def _register_op():
    """Fused step: page 0 -> state update, page 1 -> eps = x2 - S."""
    global _OP
    if _OP is not None:
        return _OP
    from concourse import dve_ops
    from concourse.dve_spec import (
        Spec, Src0, Src1, C0, C1, SubIdx, Zero, select, lower,
    )
    from concourse.dve_uop import DveOpSpec

    name = "SVS_STEP_EPS_ANT"
    for o in dve_ops.OPS:
        if o.name == name:
            _OP = o
            return o

    def _ref(in0, in1, c0, c1, c2):
        u = (in0 - in1).astype(np.float32)
        chain = np.where(u > 0, (in1 + np.float32(c0)).astype(np.float32),
                         (in1 - np.float32(c1)).astype(np.float32))
        out = chain.astype(np.float32)
        out[:, 1, :] = u[:, 1, :]
        return out

    u = Src0 - Src1
    spec = Spec(
        body=select(SubIdx, u, Src1 + select(Zero < u, C0, Zero - C1)),
        reference=_ref,
    )
    opcode = dve_ops._CUSTOM_DVE_ROW_BASE + len(dve_ops.OPS)
    shas = {}
    for ver in ("v3", "v4"):
        uops = lower(spec, ver=ver)
        shas[ver] = DveOpSpec(name=name, opcode=opcode, uops=uops, rd1_en=True).sha(ver)
    op = dve_ops.DveOp(name, spec, subdim=True, uops_sha=shas)
    dve_ops.OPS.append(op)
    dve_ops._SUB_OPCODE_FOR_NAME[name] = opcode
    dve_ops.CUSTOM_DVE_SPECS[name] = spec
    _OP = op
    return op


def _build_program(d_open, d_close, hc, sig_bias, relu_bias):
    from concourse import mybir, bacc

    op = _register_op()
    nc = bacc.Bacc("TRN2", target_bir_lowering=False, debug=False,
                   num_devices=NCORES)
    f32 = mybir.dt.float32
    f16 = mybir.dt.float16
    Sig = mybir.ActivationFunctionType.Sigmoid
    Relu = mybir.ActivationFunctionType.Relu

    xp_d = nc.dram_tensor("xp", [P, T, FD], f32, kind="ExternalInput").ap()
    s0_d = nc.dram_tensor("s0", [P, FD], f32, kind="ExternalInput").ap()
    band_d = nc.dram_tensor("band", [3, P, P], f16, kind="ExternalInput").ap()
    out_d = nc.dram_tensor("out", [P, T, STRIDE], f32, kind="ExternalOutput").ap()

    # SBUF
    x2b = nc.alloc_sbuf_tensor("x2b", [P, 3 * WIN * FD], f32).ap()
    trajr = nc.alloc_sbuf_tensor("trajr", [P, RS * SLOT], f32).ap()
    bands = nc.alloc_sbuf_tensor("bands", [P, 3 * P], f16).ap()
    thr = nc.alloc_sbuf_tensor("thr", [P, TH_RING * CE * FD], f16).ap()
    outw = nc.alloc_sbuf_tensor("outw", [P, 2 * STRIDE * WIN], f32).ap()
    # PSUM
    psb = [nc.alloc_psum_tensor(f"psb{i}", [P, CH * STRIDE], f32).ap()
           for i in range(PS_RING)]

    # semaphores
    sem_const = nc.alloc_semaphore("sem_const")  # bands+s0 DMAs (2 x 16)
    sem_x2 = [nc.alloc_semaphore("sem_x2a"),
              nc.alloc_semaphore("sem_x2b"),
              nc.alloc_semaphore("sem_x2c")]     # x2 buffer mod-3
    sem_chain = nc.alloc_semaphore("sem_chain")  # +1 per 10-step chain chunk
    sem_sig = nc.alloc_semaphore("sem_sig")      # +1 per sigmoid chunk
    sem_ps = nc.alloc_semaphore("sem_ps")        # +1 per conv chunk (TensorE)
    sem_relu = nc.alloc_semaphore("sem_relu")    # +1 per relu chunk (ScalarE)
    sem_out = [nc.alloc_semaphore("sem_outa"),
               nc.alloc_semaphore("sem_outb")]   # out DMA parity
    sem_cb = nc.alloc_semaphore("sem_cb")        # bias memsets done
    sem_s0 = nc.alloc_semaphore("sem_s0")        # s0 DMA done

    sig_bias_t = nc.alloc_sbuf_tensor("sig_bias_t", [P, 1], f32).ap()
    relu_bias_t = nc.alloc_sbuf_tensor("relu_bias_t", [P, 1], f32).ap()
    nc.gpsimd.memset(sig_bias_t[:], sig_bias)
    nc.gpsimd.memset(relu_bias_t[:], relu_bias).then_inc(sem_cb, 16)
    nc.scalar.wait_ge(sem_cb, 16)

    # window bookkeeping
    t0s = []
    t0 = 0
    for wlen in WINS:
        t0s.append(t0)
        t0 += wlen
    cum10 = np.cumsum([w // CE for w in WINS]).tolist()
    cum20 = np.cumsum([w // CH for w in WINS]).tolist()
    NW = len(WINS)
    NC20 = cum20[-1]

    x2v = x2b.rearrange("p (b t f) -> p b t f", b=3, f=FD)       # [P,3,WIN,FD]
    trajv = trajr.rearrange("p (s h f) -> p s h f", s=RS, f=FD)  # [P,RS,2,FD]
    thv = thr.rearrange("p (r t f) -> p r t f", r=TH_RING, f=FD)
    outwv = outw.rearrange("p (b j) -> p b j", b=2)

    # ---- initial DMAs (sync queue): chain dependencies (s0, x2 w0) first,
    # bands (needed ~10us later by the first conv) after ----
    # s0 lives in the state half of ring slot RS-1 (step 0 reads it there)
    nc.sync.dma_start(trajv[:, RS - 1, 0, :], s0_d[:]).then_inc(sem_s0, 16)

    def x2_dma(w):
        wlen = WINS[w]
        if w >= 3:
            # x2 buffer WAR: the chain (its only reader) is done with window
            # w-3 once sigmoid is, and gating on sigmoid also bounds how far
            # the chain can run ahead of the eps consumers (ring overrun)
            nc.sync.wait_ge(sem_sig, cum10[w - 3])
        nc.sync.dma_start(
            x2v[:, w % 3, 0:wlen, :],
            xp_d[:, t0s[w]:t0s[w] + wlen, :],
        ).then_inc(sem_x2[w % 3], 16)

    x2_dma(0)
    nc.sync.dma_start(bands[:].rearrange("p (d q) -> p d q", d=3),
                      band_d.rearrange("d p q -> p d q")).then_inc(sem_const, 16)
    x2_dma(1)
    x2_dma(2)

    # ---- engine emission helpers ----
    def chain_window(w):
        wlen = WINS[w]
        t0w = t0s[w]
        if w == 0:
            nc.vector.wait_ge(sem_s0, 16)
        nc.vector.wait_ge(sem_x2[w % 3], 16 * (w // 3 + 1))
        for i in range(wlen):
            t = t0w + i
            sprev = (t - 1) % RS
            inst = nc.vector._custom_dve(
                op,
                out=trajv[:, t % RS, :, :],
                in0=x2v[:, w % 3, i, :].unsqueeze(1).broadcast_to((P, 2, FD)),
                in1=trajv[:, sprev, 0, :].unsqueeze(1).broadcast_to((P, 2, FD)),
                s0=d_open,
                s1=d_close,
            )
            if (t + 1) % CE == 0:
                inst.then_inc(sem_chain, 1)

    def sig_chunk(k):
        # eps pages of steps [10k, 10k+10) live at ring slots 10k..10k+9
        s1 = (CE * k) % RS
        nc.scalar.wait_ge(sem_chain, k + 1)
        if k >= TH_RING:
            nc.scalar.wait_ge(sem_ps, (k - TH_RING) // 2 + 1)
        nc.scalar.activation(
            thv[:, k % TH_RING, :, :], trajv[:, s1:s1 + CE, 1, :],
            Sig, bias=sig_bias_t[:], scale=hc,
        ).then_inc(sem_sig, 1)

    def conv_chunk(j):
        # steps [CH*j, CH*j+CH) = th chunks 2j, 2j+1 (contiguous ring slots)
        if j == 0:
            nc.tensor.wait_ge(sem_const, 16)
        nc.tensor.wait_ge(sem_sig, 2 * j + 2)
        if j >= PS_RING:
            nc.tensor.wait_ge(sem_relu, j - PS_RING + 1)
        ps = psb[j % PS_RING][:].rearrange("p (t f) -> p t f", t=CH)
        h0 = (2 * j) % TH_RING
        th2 = thr[:, h0 * CE * FD:(h0 + 2) * CE * FD].rearrange(
            "p (t f) -> p t f", f=FD)
        m = 0
        for dx in (-1, 0, 1):
            for half in (0, PW):
                o = half + 1 + dx
                inst = nc.tensor.matmul(
                    ps, bands[:, (dx + 1) * P:(dx + 2) * P],
                    th2[:, :, o:o + STRIDE],
                    start=(m == 0), stop=(m == 5),
                )
                m += 1
        inst.then_inc(sem_ps, 1)

    def relu_chunk(j):
        w = next(i for i, c in enumerate(cum20) if j < c)
        first_of_w = (j == 0) or (j == cum20[w - 1])
        if first_of_w and w >= 2:
            nc.scalar.wait_ge(sem_out[w % 2], 16 * (w // 2))
        nc.scalar.wait_ge(sem_ps, j + 1)
        off = (CH * j - t0s[w]) * STRIDE
        nc.scalar.activation(
            outwv[:, w % 2, off:off + CH * STRIDE], psb[j % PS_RING][:],
            Relu, bias=relu_bias_t[:], scale=1.0,
        ).then_inc(sem_relu, 1)

    def out_dma(w):
        wlen = WINS[w]
        nc.sync.wait_ge(sem_relu, cum20[w])
        nc.sync.dma_start(
            out_d[:, t0s[w]:t0s[w] + wlen, :],
            outwv[:, w % 2, 0:STRIDE * wlen].rearrange(
                "p (t f) -> p t f", f=STRIDE),
        ).then_inc(sem_out[w % 2], 16)

    # ---- emit (order matters only within each engine's own queue) ----
    for w in range(NW):
        chain_window(w)
        k_lo = 0 if w == 0 else cum10[w - 1]
        for k in range(k_lo, cum10[w]):
            sig_chunk(k)
            if k % 2 == 1:
                conv_chunk((k - 1) // 2)
                # relu j lands two sigmoid chunks after conv j so ScalarE
                # doesn't stall on the PE
                if k >= 3:
                    relu_chunk((k - 3) // 2)
        if w + 3 < NW:
            x2_dma(w + 3)
        out_dma(w)
    relu_chunk(NC20 - 1)

    nc.sync.wait_ge(sem_out[0], 16 * ((NW + 1) // 2))
    nc.sync.wait_ge(sem_out[1], 16 * (NW // 2))
    nc.all_engine_barrier()
    nc.compile()
    return nc


_PROG_CACHE = {}


def _get_program(key, d_open, d_close, hc, sig_bias, relu_bias):
    if key not in _PROG_CACHE:
        _PROG_CACHE[key] = _build_program(d_open, d_close, hc, sig_bias,
                                          relu_bias)
    return _PROG_CACHE[key]


def _prep_inputs(x, ht0, lt0, kern, hc):
    x = np.ascontiguousarray(x.reshape(T, H, W).astype(np.float32))
    ht0 = ht0.astype(np.float32)
    lt0 = lt0.astype(np.float32)
    kern = kern.astype(np.float32)

    xp = np.full((T, H + 2, W + 3), PAD, np.float32)
    xp[:, 1:H + 1, 1:W + 1] = x
    hp = np.zeros((H + 2, W + 3), np.float32)
    hp[1:H + 1, 1:W + 1] = ht0
    lp = np.zeros((H + 2, W + 3), np.float32)
    lp[1:H + 1, 1:W + 1] = -lt0

    band = np.zeros((3, P, P), np.float16)
    for dxi in range(3):
        for h_out in range(ROWS):
            for dy in (-1, 0, 1):
                h_in = h_out + dy
                if 0 <= h_in < ROWS:
                    for wb in range(NB):
                        band[dxi, h_in * NB + wb, h_out * NB + wb] = \
                            kern[dy + 1, dxi]

    in_maps = []
    for c in range(NCORES):
        r0 = 16 * c
        xc = np.empty((ROWS, NB, T, FD), np.float32)
        sc = np.empty((ROWS, NB, FD), np.float32)
        for wb in range(NB):
            c0 = STRIDE * wb
            blk = xp[:, r0:r0 + ROWS, c0:c0 + PW].transpose(1, 0, 2)
            xc[:, wb, :, 0:PW] = blk
            xc[:, wb, :, PW:FD] = -blk
            sc[:, wb, 0:PW] = hp[r0:r0 + ROWS, c0:c0 + PW]
            sc[:, wb, PW:FD] = lp[r0:r0 + ROWS, c0:c0 + PW]
        in_maps.append({
            "xp": np.ascontiguousarray(xc.reshape(P, T, FD)),
            "s0": np.ascontiguousarray(sc.reshape(P, FD)),
            "band": band,
        })
    return in_maps


TRACE = False
LAST_RESULT = None


def kernel(x, params, ht0, lt0, kernel):
    global LAST_RESULT
    from concourse.bass_utils import run_bass_kernel_spmd

    p = np.asarray(params, np.float32)
    d_close, d_open, d_hot, hc = (float(p[0]), float(p[1]), float(p[2]),
                                  float(p[3]))
    kern = np.asarray(kernel, np.float32)
    sig_bias = float(np.float32(-np.float32(d_hot) * np.float32(hc)))
    relu_bias = float(np.float32(1.0) - np.float32(kern.sum()))

    key = (d_close, d_open, d_hot, hc, kern.tobytes())
    nc = _get_program(key, d_open, d_close, hc, sig_bias, relu_bias)
    in_maps = _prep_inputs(np.asarray(x), np.asarray(ht0), np.asarray(lt0),
                           kern, hc)
    r = run_bass_kernel_spmd(nc, in_maps, list(range(NCORES)), trace=TRACE)
    LAST_RESULT = r
    res = r.results
    out = np.empty((T, H, W), np.float32)
    for c in range(NCORES):
        out[:, 16 * c:16 * (c + 1), :] = _assemble(res[c]["out"])
    return out.reshape(T, 1, H, W).astype(np.float32)


def _assemble(raw):
    v = raw.reshape(ROWS, NB, T, STRIDE)[1:17]
    full = v.transpose(2, 0, 1, 3).reshape(T, 16, NB * STRIDE)
    return full[:, :, :W]
